# revision 36
# baseline (speedup 1.0000x reference)
"""Trainium2 Bass kernel for a 2-layer dense-adjacency GAT (nn_GAT_17824114278677).

Sharding: nodes (rows of the attention matrix) are sharded across the 8
NeuronCores, 512 rows per core; weights and node features are replicated.
Two SPMD launches (one per GAT layer) with a host-side gather of the layer-1
output in between.

Per-core dataflow: attention tiles are computed TRANSPOSED, [j=128
partitions, r=512 rows], so the aggregation att @ Wh maps directly onto the
PE (contraction over j on partitions) with zero on-chip transposes.

Math: softmax is invariant to per-row scaling, so the row factor
u_i = exp(f_src_i) cancels; per-key factors fold into the stationary
operand.  From exp(leaky_relu(t)) = max(exp(t), exp(0.2 t)):

    att[j, i]  (up to a row factor)
      = v_j * m01[j, i] * max(1, g_i * w_j)        g = exp(-0.8 f_src)
      = (v_j w_j) * m01[j, i] * max(iw_j, g_i)     w = exp(-0.8 f_dst)
                                                   iw = 1/w, v = exp(f_dst)

so with the stationary Whx2 = Wh * exp(0.2 f_dst) (and an exp(0.2 f_dst)
denominator column), the per-element work is exactly

    z = max(g_i, iw_j)   (one tensor_scalar, op0=max, per-partition scalar)
    p = z * m01          (one tensor_tensor)

Masked entries are exactly 0, matching the reference's -9e15 mask.  Chunk
groups are routed per-style to balance all four engines:

  V: DVE tensor_scalar z + DVE tensor_tensor p.
  A: ScalarE s = relu(g - iw) (per-partition bias AP) + DVE tensor_tensor
     p = s*m; the missing "+m" term rides the PE as an extra accumulating
     matmul of the raw mask against Whx1 = Wh * exp(f_dst)
     (m*max(iw,g)*w*v = m*v + m*relu(g-iw)*exp(.2 fd)).
  P: DVE tensor_scalar z + GpSimd (Pool) tensor_tensor p.

Softmax denominators ride as an extra stationary column; division + ELU
happen on the host on the tiny per-head [HID+1, 512] outputs.
"""

import os
import sys
import time
from contextlib import ExitStack

for _p in ("/opt/trn_rl_repo", "/root/.axon_site/_ro/trn_rl_repo"):
    if os.path.isdir(_p) and _p not in sys.path:
        sys.path.append(_p)

import numpy as np
import ml_dtypes

import bass_rust
import concourse.bass as bass
import concourse.tile as tile
from concourse import mybir
from concourse.bass_utils import run_bass_kernel_spmd

BF16 = ml_dtypes.bfloat16
F32 = mybir.dt.float32
BF = mybir.dt.bfloat16
F8 = mybir.dt.float8e4
F8NP = mybir.dt.np(F8)
DROW = mybir.MatmulPerfMode.DoubleRow

N = 4096          # nodes
NCORES = 8
R = N // NCORES   # rows (queries) per core
CJ = N // 128     # 32 key chunks
FIN = 512         # input feature dim of both layers
GRP = 4           # chunk-group size for fused VectorE/Pool ops

# Per-head style schedule for the CJ//GRP = 8 chunk groups.
# 'V' = DVE-only, 'A' = ScalarE relu + DVE mask (+ extra PE matmul),
# 'P' = DVE tensor_scalar + Pool(GpSimd) mask multiply.
STY1 = ("A", "V", "A", "V", "A", "V", "A", "A")   # layer 1
STY2 = ("A", "V", "A", "V", "A", "V", "V", "A")   # layer 2

CORE_IDS = list(range(NCORES))

LAST_PERF = {}


# ---------------------------------------------------------------------------
# walrus workaround: it rejects instructions carrying >1 sync-wait command
# ("Too many sync wait commands").  Move excess waits onto preceding
# same-engine NoOps -- semantically identical (same-engine waits are totally
# ordered before the instruction).
def _split_excess_waits(nc, max_waits: int = 1) -> int:
    n_split = 0
    for fn in nc.m.functions:
        for bb in fn.blocks:
            insts = bb.instructions
            new_insts = []
            changed = False
            for ins in insts:
                si = ins.sync_info
                waits = list(si.on_wait) if si is not None else []
                if len(waits) > max_waits:
                    extra, keep = waits[:-max_waits], waits[-max_waits:]
                    for k in range(0, len(extra), max_waits):
                        chunk = extra[k : k + max_waits]
                        nop = bass_rust.InstNoOp(
                            name=f"{ins.name}-wsplit{k}", ins=[], outs=[]
                        )
                        nop.engine = ins.engine
                        nop.sync_info = mybir.SyncInfo(on_wait=chunk, on_update=[])
                        new_insts.append(nop)
                        n_split += 1
                    si.on_wait = keep
                    changed = True
                new_insts.append(ins)
            if changed:
                bb.instructions = new_insts
    return n_split


def _a_chunks(styles):
    """Absolute chunk indices covered by 'A' style groups."""
    out = []
    for gi, s in enumerate(styles):
        if s == "A":
            out.extend(range(gi * GRP, gi * GRP + GRP))
    return out


# ---------------------------------------------------------------------------
def _build_layer(H: int, HID: int, styles=STY1):
    """One GAT layer, per-core program.

    Inputs (per core):
      whxin  [128, CJ, H, WPH] bf16  Wh*exp(.2 fd) per head + exp(.2 fd) col
      whxa   [128, nA, H, WPH] bf16  Wh*exp(fd) + exp(fd) col, A-chunks only
      maskM  [128, CJ, R]      bf16  0/1 adjacency, chunk-major, transposed
      gBin   [128, H, R]       bf16  exp(-0.8 f_src) of this core's rows,
                                     broadcast along partitions
      iwcol  [128, H*CJ]       f32   exp(0.8 f_dst), [p, h*CJ+c] = iw[h, 128c+p]
      niwcol [128, H*CJ]       f32   -exp(0.8 f_dst)
    Output:
      agg    [H, HID+1, R]  f32   rows 0..HID-1: unnormalized transposed
                                  numerator; row HID: softmax denominator
    """
    WPH = HID + 2  # per-head stride in whx: HID cols + denom col + pad
    # whxa per-head stride padded to a 16-byte multiple so every per-head
    # slice offset and the DoubleRow pair step meet the s3_lw dual-fp8
    # alignment restrictions
    WPHA = HID + 2
    while WPHA % 16:
        WPHA += 1
    ach = _a_chunks(styles)
    a_ix = {c: i for i, c in enumerate(ach)}
    nA = max(1, len(ach))

    nc = bass.Bass("TRN2", debug=False, num_devices=NCORES)
    whxin = nc.dram_tensor("whxin", [128, CJ, H, WPH], BF, kind="ExternalInput")
    whxa = nc.dram_tensor("whxa", [128, nA, H, WPHA], F8, kind="ExternalInput")
    maskM = nc.dram_tensor("maskM", [128, CJ, R], BF, kind="ExternalInput")
    mask8 = nc.dram_tensor("mask8", [128, CJ, R], F8, kind="ExternalInput")
    gBin = nc.dram_tensor("gBin", [128, H, R], BF, kind="ExternalInput")
    iwcol = nc.dram_tensor("iwcol", [128, H * CJ], F32, kind="ExternalInput")
    niwcol = nc.dram_tensor("niwcol", [128, H * CJ], F32, kind="ExternalInput")
    agg = nc.dram_tensor("agg", [H, HID + 1, R], F32, kind="ExternalOutput")

    RELU = mybir.ActivationFunctionType.Relu
    MAX = mybir.AluOpType.max
    MUL = mybir.AluOpType.mult

    groups = []
    for gi in range(CJ // GRP):
        groups.append((gi * GRP, GRP, styles[gi]))

    with tile.TileContext(nc) as tc, ExitStack() as ctx:
        cpool = ctx.enter_context(tc.tile_pool(name="const", bufs=1))
        wpool = ctx.enter_context(tc.tile_pool(name="whx", bufs=1))
        spool = ctx.enter_context(tc.tile_pool(name="srelu", bufs=2))
        zpool = ctx.enter_context(tc.tile_pool(name="zmax", bufs=4))
        ppool = ctx.enter_context(tc.tile_pool(name="p3", bufs=4))
        opool = ctx.enter_context(tc.tile_pool(name="out", bufs=2))
        paq = ctx.enter_context(
            tc.tile_pool(name="psa", bufs=min(H, 8), space="PSUM")
        )

        # ---- resident constants -------------------------------------------
        # issue order matters: iw/niw/gB feed the first compute; the mask
        # parts and whx chunks stream behind.
        iw_t = cpool.tile([128, H * CJ], F32, tag="iwcol")
        nc.sync.dma_start(iw_t[:], iwcol[:])
        niw_t = cpool.tile([128, H * CJ], F32, tag="niwcol")
        nc.sync.dma_start(niw_t[:], niwcol[:])
        g_t = cpool.tile([128, H, R], BF, tag="gB")
        nc.sync.dma_start(g_t[:], gBin[:])

        whxa_t = cpool.tile([128, nA, H, WPHA], F8, tag="whxa")
        nc.sync.dma_start(whxa_t[:], whxa[:])

        # interleave the fp8 mask, bf16 mask, and stationary parts so early
        # chunks of all three land first
        m8_t = cpool.tile([128, CJ, R], F8, tag="mask8")
        mask_t = cpool.tile([128, CJ, R], BF, tag="mask")
        whx_t = wpool.tile([128, CJ, H, WPH], BF, tag="whxall")
        NMQ = 8
        for mq in range(NMQ):
            cs = slice(mq * (CJ // NMQ), (mq + 1) * (CJ // NMQ))
            nc.sync.dma_start(m8_t[:, cs, :], mask8[:, cs, :])
            nc.sync.dma_start(mask_t[:, cs, :], maskM[:, cs, :])
            nc.sync.dma_start(whx_t[:, cs], whxin[:, cs])

        n_extra = sum(G // 2 for _, G, s in groups if s == "A")
        total_mm = CJ + n_extra

        # tt consumption order: P groups early (Pool gets going), A groups
        # spread mid (ScalarE results arrive while DVE does V work).
        p_g = [t for t in groups if t[2] == "P"]
        a_g = [t for t in groups if t[2] == "A"]
        v_g = [t for t in groups if t[2] == "V"]
        tt_order = []
        qs = [p_g, a_g, v_g]
        while any(qs):
            for q in qs:
                if q:
                    tt_order.append(q.pop(0))

        # ---- attention + aggregation --------------------------------------
        # One PSUM bank per head, all live at once.  Each head's A-style "+m"
        # matmuls (fp8 DoubleRow pairs: rhs [128, 2, R], lhsT [128, 2, HID+1]
        # contracting TWO mask chunks at half the row cost) issue at the head
        # start: they only need DMA-landed data, so they fill what would be
        # PE idle gaps between heads and keep the HAM throttle warm.
        pas = [
            paq.tile([HID + 1, R], F32, tag="psa", name=f"pa{h}")
            for h in range(H)
        ]

        for h in range(H):
            pa = pas[h]
            mm = 0
            for c0, G, sty in groups:
                if sty != "A":
                    continue
                for k in range(0, G, 2):
                    c = c0 + k
                    # weights AP [Ki, Ko=2, M]: pair at dim 1 with a 16B-
                    # aligned step (BIR + s3_lw dual-fp8 restrictions)
                    nc.tensor.matmul(
                        pa[:],
                        whxa_t[:, a_ix[c] : a_ix[c] + 2, h, 0 : HID + 1],
                        m8_t[:, c : c + 2, :],
                        start=(mm == 0), stop=False,
                        perf_mode=DROW,
                    )
                    mm += 1

            # A-style ScalarE tiles: s = relu(g_i - iw_j)
            s_tiles = {}
            for c0, G, sty in groups:
                if sty != "A":
                    continue
                s = spool.tile([128, GRP, R], BF, tag="sa")
                for k in range(G):
                    ix = h * CJ + c0 + k
                    nc.scalar.activation(
                        s[:, k, :], g_t[:, h, :], RELU,
                        bias=niw_t[:, ix : ix + 1], scale=1.0,
                    )
                s_tiles[c0] = s

            # z = max(g_i, iw_j): DVE tensor_scalar (V and P styles)
            z_tiles = {}
            for c0, G, sty in groups:
                if sty == "A":
                    continue
                z = zpool.tile([128, GRP, R], BF, tag="zm")
                for k in range(G):
                    ix = h * CJ + c0 + k
                    nc.vector.tensor_scalar(
                        z[:, k, :], g_t[:, h, :],
                        iw_t[:, ix : ix + 1], None, op0=MAX,
                    )
                z_tiles[c0] = z

            # mask multiplies (DVE grouped; GpSimd NEVER touches these --
            # concurrent Q7 SBUF traffic halves DVE throughput) + matmuls.
            # The head's last two tt groups are split in half so the PE's
            # tail wait stays under the ~3.4us HAM re-throttle window.
            for gi, (c0, G, sty) in enumerate(tt_order):
                p3 = ppool.tile([128, GRP, R], BF, tag="p3")
                src = s_tiles[c0] if sty == "A" else z_tiles[c0]
                hg = G // 2
                subs = (
                    [(0, hg), (hg, G - hg)]
                    if (gi >= len(tt_order) - 2 and hg)
                    else [(0, G)]
                )
                for k0, gn in subs:
                    nc.vector.tensor_tensor(
                        p3[:, k0 : k0 + gn, :], src[:, k0 : k0 + gn, :],
                        mask_t[:, c0 + k0 : c0 + k0 + gn, :], op=MUL,
                    )
                    for k in range(k0, k0 + gn):
                        c = c0 + k
                        nc.tensor.matmul(
                            pa[:], whx_t[:, c, h, 0 : HID + 1], p3[:, k, :],
                            start=(mm == 0), stop=(mm == total_mm - 1),
                        )
                        mm += 1

            o = opool.tile([HID + 1, R], F32, tag="aggo")
            nc.scalar.copy(o[:], pa[:])
            nc.sync.dma_start(agg[h], o[:])

    return nc


_PROGS = {}


def _get_prog(H, HID, styles):
    """Build (and cache) the layer program with the walrus wait-split fix
    applied.  The fix is HW-only: CoreSim's event loop rejects the injected
    NoOps, so sim users should call _build_layer directly."""
    key = (H, HID, styles)
    if key not in _PROGS:
        nc = _build_layer(H, HID, styles)
        _split_excess_waits(nc)
        _PROGS[key] = nc
    return _PROGS[key]


def _elu(v):
    return np.where(v > 0, v, np.expm1(np.minimum(v, 0.0))).astype(np.float32)


def _col_layout(x, H):
    """[N, H] -> [128, H*CJ] with [p, h*CJ+c] = x[128c+p, h]."""
    return np.ascontiguousarray(
        x.T.reshape(H, CJ, 128).transpose(2, 0, 1).reshape(128, H * CJ)
    ).astype(np.float32)


def _whx_layout(Wh, col_scale, H, HID, chunks=None, dtype=BF16, wph=None):
    """Stationary tensor: [128, CJ', H, WPH] with per-head [HID] features
    scaled by col_scale plus a col_scale denominator column."""
    WPH = wph if wph is not None else HID + 2
    scaled = (Wh.reshape(N, H, HID) * col_scale[:, :, None]).astype(np.float32)
    full = np.zeros((128, CJ, H, WPH), np.float32)
    full[:, :, :, :HID] = scaled.reshape(CJ, 128, H, HID).transpose(1, 0, 2, 3)
    full[:, :, :, HID] = col_scale.reshape(CJ, 128, H).transpose(1, 0, 2)
    if chunks is not None:
        full = full[:, chunks]
    return np.ascontiguousarray(full).astype(dtype)


def _host_inputs(f_src, f_dst, adj, Wh, H, styles):
    """Shared per-layer host prep.  f_src/f_dst [N, H] f32, adj [N, N] i32,
    Wh [N, H*HID] f32 (pre-activation per-head features)."""
    HID = Wh.shape[1] // H
    wpha = HID + 2
    while wpha % 16:
        wpha += 1
    iw = np.exp(0.8 * f_dst).astype(np.float32)       # [N, H]
    iw_arr = _col_layout(iw, H)

    ach = _a_chunks(styles)
    shared = {
        "iwcol": iw_arr,
        "niwcol": -iw_arr,
        # V/P stationaries: Wh * exp(.2 fd) (= Wh * v * w)
        "whxin": _whx_layout(Wh, np.exp(0.2 * f_dst).astype(np.float32), H, HID),
        # A-style "+m" stationaries: Wh * exp(fd) (= Wh * v), A-chunks only,
        # fp8 for DoubleRow paired matmuls (stride padded to 16B multiples)
        "whxa": _whx_layout(
            Wh, np.exp(f_dst).astype(np.float32), H, HID,
            chunks=ach if ach else [0], dtype=F8NP, wph=wpha,
        ),
    }
    g8 = np.exp(-0.8 * f_src).astype(np.float32)  # [N, H]
    per_core = []
    for i in range(NCORES):
        rows = slice(R * i, R * (i + 1))
        adjT = adj[rows, :].T.astype(np.float32)  # [N, R] 0/1
        d = dict(shared)
        mm_arr = np.ascontiguousarray(adjT.reshape(CJ, 128, R).transpose(1, 0, 2))
        d["maskM"] = mm_arr.astype(BF16)
        d["mask8"] = mm_arr.astype(F8NP)
        gs = np.ascontiguousarray(g8[rows, :].T)  # [H, R]
        d["gBin"] = np.broadcast_to(gs[None, :, :], (128, H, R)).astype(BF16)
        per_core.append(d)
    return per_core


def _run_layer(nc, in_maps, H, HID, tag):
    t0 = time.time()
    res = run_bass_kernel_spmd(nc, in_maps, core_ids=CORE_IDS)
    LAST_PERF[f"{tag}_wall_s"] = time.time() - t0
    LAST_PERF[f"{tag}_exec_ns"] = res.exec_time_ns

    hT = np.empty((H * HID, N), np.float32)
    for i in range(NCORES):
        a = res.results[i]["agg"]  # [H, HID+1, R]
        denom = a[:, HID : HID + 1, :]
        hT[:, R * i : R * (i + 1)] = (a[:, :HID, :] / denom).reshape(H * HID, R)
    return hT


def kernel(x, adj, W1, a1, W2, a2):
    x = np.asarray(x, np.float32)
    adj = np.asarray(adj, np.int32)
    W1 = np.asarray(W1, np.float32)
    a1 = np.asarray(a1, np.float32)
    W2 = np.asarray(W2, np.float32)
    a2 = np.asarray(a2, np.float32)

    H1, HID1, OUT = W1.shape[0], W1.shape[2], W2.shape[1]

    progA = _get_prog(H1, HID1, STY1)
    progB = _get_prog(1, OUT, STY2)

    # ---- layer 1 ----------------------------------------------------------
    W1c = np.ascontiguousarray(W1.transpose(1, 0, 2).reshape(FIN, H1 * HID1))
    wsrc1 = np.einsum("hfk,hk->fh", W1, a1[:, :HID1, 0]).astype(np.float32)
    wdst1 = np.einsum("hfk,hk->fh", W1, a1[:, HID1:, 0]).astype(np.float32)
    f_src1 = x @ wsrc1  # [N, H]
    f_dst1 = x @ wdst1
    Wh1 = x @ W1c  # [N, H1*HID1]

    in_maps = _host_inputs(f_src1, f_dst1, adj, Wh1, H1, STY1)
    hT = _run_layer(progA, in_maps, H1, HID1, "layer1")
    hcatT = _elu(hT)  # [512, N] == h_cat.T (concat=True applies elu)

    # ---- layer 2 ----------------------------------------------------------
    hcat = np.ascontiguousarray(hcatT.T)  # [N, 512]
    wsrc2 = (W2 @ a2[:OUT, 0]).astype(np.float32)[:, None]
    wdst2 = (W2 @ a2[OUT:, 0]).astype(np.float32)[:, None]
    f_src2 = hcat @ wsrc2  # [N, 1]
    f_dst2 = hcat @ wdst2
    Wh2 = hcat @ W2  # [N, OUT]
    in_maps2 = _host_inputs(f_src2, f_dst2, adj, Wh2, 1, STY2)
    outT = _run_layer(progB, in_maps2, 1, OUT, "layer2")
    # layer 2: concat=False -> no inner elu; final output = elu(out)
    return np.ascontiguousarray(_elu(outT).T)


# revision 40
# speedup vs baseline: 1.0236x; 1.0236x over previous
"""Trainium2 Bass kernel for a 2-layer dense-adjacency GAT (nn_GAT_17824114278677).

Sharding: nodes (rows of the attention matrix) are sharded across the 8
NeuronCores, 512 rows per core; weights and node features are replicated.
Two SPMD launches (one per GAT layer) with a host-side gather of the layer-1
output in between.

Per-core dataflow: attention tiles are computed TRANSPOSED, [j=128
partitions, r=512 rows], so the aggregation att @ Wh maps directly onto the
PE (contraction over j on partitions) with zero on-chip transposes.

Math: softmax is invariant to per-row scaling, so the row factor
u_i = exp(f_src_i) cancels; per-key factors fold into the stationary
operand.  From exp(leaky_relu(t)) = max(exp(t), exp(0.2 t)):

    att[j, i]  (up to a row factor)
      = v_j * m01[j, i] * max(1, g_i * w_j)        g = exp(-0.8 f_src)
      = (v_j w_j) * m01[j, i] * max(iw_j, g_i)     w = exp(-0.8 f_dst)
                                                   iw = 1/w, v = exp(f_dst)

so with the stationary Whx2 = Wh * exp(0.2 f_dst) (and an exp(0.2 f_dst)
denominator column), the per-element work is exactly

    z = max(g_i, iw_j)   (one tensor_scalar, op0=max, per-partition scalar)
    p = z * m01          (one tensor_tensor)

Masked entries are exactly 0, matching the reference's -9e15 mask.  Chunk
groups are routed per-style to balance the engines (HW-measured rates):

  V: DVE tensor_scalar z (~347ns/[128,512]) + DVE tensor_tensor p
     (~303ns/chunk grouped x4).
  A: ScalarE s = relu(g - iw) (per-partition bias AP, ~706ns) + DVE
     tensor_tensor p = s*m; the missing "+m" term rides the PE as fp8e4
     DoubleRow PAIRED matmuls of the raw mask against Whx1 = Wh*exp(f_dst)
     (m*max(iw,g)*w*v = m*v + m*relu(g-iw)*exp(.2 fd)); a pair of mask
     chunks costs 421ns (vs 379ns/chunk for normal bf16 matmuls).

GpSimd is left COMPLETELY idle on purpose: its Q7 software tensor ops
share SBUF ports with the DVE and degrade concurrent DVE throughput ~4x.
Per-head PSUM banks (all 8 live) let each head's mask-matmuls issue at
head start, filling PE idle windows so the HAM throttle stays warm.

Softmax denominators ride as an extra stationary column; division + ELU
happen on the host on the tiny per-head [HID+1, 512] outputs.

Measured on 8 axon-tunneled trn2 cores: ~140.5us (layer 1) + ~44.8us
(layer 2) = ~185.3us total, rel err ~5.9e-3 vs the fp32 jax reference
(fp8 mask-matmul terms cost ~5e-3; the bf16-only variant measures 8.3e-4
at ~264us).  Layer 1 runs with the PE at ~95% of span (379ns warm
matmuls), DVE ~92%, ScalarE ~74%.
"""

import os
import sys
import time
from contextlib import ExitStack

for _p in ("/opt/trn_rl_repo", "/root/.axon_site/_ro/trn_rl_repo"):
    if os.path.isdir(_p) and _p not in sys.path:
        sys.path.append(_p)

import numpy as np
import ml_dtypes

import bass_rust
import concourse.bass as bass
import concourse.tile as tile
from concourse import mybir
from concourse.bass_utils import run_bass_kernel_spmd

BF16 = ml_dtypes.bfloat16
F32 = mybir.dt.float32
BF = mybir.dt.bfloat16
F8 = mybir.dt.float8e4
F8NP = mybir.dt.np(F8)
DROW = mybir.MatmulPerfMode.DoubleRow

N = 4096          # nodes
NCORES = 8
R = N // NCORES   # rows (queries) per core
CJ = N // 128     # 32 key chunks
FIN = 512         # input feature dim of both layers
GRP = 4           # chunk-group size for fused VectorE/Pool ops

# Per-head style schedule for the CJ//GRP = 8 chunk groups.
# 'V' = DVE-only, 'A' = ScalarE relu + DVE mask (+ extra PE matmul),
# 'P' = DVE tensor_scalar + Pool(GpSimd) mask multiply.
STY1 = ("A", "V", "A", "V", "A", "V", "A", "V")   # layer 1
STY2 = ("A", "V", "V", "V", "A", "V", "V", "V")   # layer 2

CORE_IDS = list(range(NCORES))

LAST_PERF = {}


# ---------------------------------------------------------------------------
# walrus workaround: it rejects instructions carrying >1 sync-wait command
# ("Too many sync wait commands").  Move excess waits onto preceding
# same-engine NoOps -- semantically identical (same-engine waits are totally
# ordered before the instruction).
def _split_excess_waits(nc, max_waits: int = 1) -> int:
    n_split = 0
    for fn in nc.m.functions:
        for bb in fn.blocks:
            insts = bb.instructions
            new_insts = []
            changed = False
            for ins in insts:
                si = ins.sync_info
                waits = list(si.on_wait) if si is not None else []
                if len(waits) > max_waits:
                    extra, keep = waits[:-max_waits], waits[-max_waits:]
                    for k in range(0, len(extra), max_waits):
                        chunk = extra[k : k + max_waits]
                        nop = bass_rust.InstNoOp(
                            name=f"{ins.name}-wsplit{k}", ins=[], outs=[]
                        )
                        nop.engine = ins.engine
                        nop.sync_info = mybir.SyncInfo(on_wait=chunk, on_update=[])
                        new_insts.append(nop)
                        n_split += 1
                    si.on_wait = keep
                    changed = True
                new_insts.append(ins)
            if changed:
                bb.instructions = new_insts
    return n_split


def _a_chunks(styles):
    """Absolute chunk indices covered by 'A' style groups."""
    out = []
    for gi, s in enumerate(styles):
        if s == "A":
            out.extend(range(gi * GRP, gi * GRP + GRP))
    return out


# ---------------------------------------------------------------------------
def _build_layer(H: int, HID: int, styles=STY1):
    """One GAT layer, per-core program.

    Inputs (per core):
      whxin  [128, CJ, H, WPH] bf16  Wh*exp(.2 fd) per head + exp(.2 fd) col
      whxa   [128, nA, H, WPH] bf16  Wh*exp(fd) + exp(fd) col, A-chunks only
      maskM  [128, CJ, R]      bf16  0/1 adjacency, chunk-major, transposed
      gBin   [128, H, R]       bf16  exp(-0.8 f_src) of this core's rows,
                                     broadcast along partitions
      iwcol  [128, H*CJ]       f32   exp(0.8 f_dst), [p, h*CJ+c] = iw[h, 128c+p]
      niwcol [128, H*CJ]       f32   -exp(0.8 f_dst)
    Output:
      agg    [H, HID+1, R]  f32   rows 0..HID-1: unnormalized transposed
                                  numerator; row HID: softmax denominator
    """
    WPH = HID + 2  # per-head stride in whx: HID cols + denom col + pad
    # whxa per-head stride padded to a 16-byte multiple so every per-head
    # slice offset and the DoubleRow pair step meet the s3_lw dual-fp8
    # alignment restrictions
    WPHA = HID + 2
    while WPHA % 16:
        WPHA += 1
    ach = _a_chunks(styles)
    a_ix = {c: i for i, c in enumerate(ach)}
    nA = max(1, len(ach))

    nc = bass.Bass("TRN2", debug=False, num_devices=NCORES)
    whxin = nc.dram_tensor("whxin", [128, CJ, H, WPH], BF, kind="ExternalInput")
    whxa = nc.dram_tensor("whxa", [128, nA, H, WPHA], F8, kind="ExternalInput")
    maskM = nc.dram_tensor("maskM", [128, CJ, R], BF, kind="ExternalInput")
    mask8 = nc.dram_tensor("mask8", [128, CJ, R], F8, kind="ExternalInput")
    gBin = nc.dram_tensor("gBin", [128, H, R], BF, kind="ExternalInput")
    iwcol = nc.dram_tensor("iwcol", [128, H * CJ], F32, kind="ExternalInput")
    niwcol = nc.dram_tensor("niwcol", [128, H * CJ], F32, kind="ExternalInput")
    agg = nc.dram_tensor("agg", [H, HID + 1, R], F32, kind="ExternalOutput")

    RELU = mybir.ActivationFunctionType.Relu
    MAX = mybir.AluOpType.max
    MUL = mybir.AluOpType.mult

    groups = []
    for gi in range(CJ // GRP):
        groups.append((gi * GRP, GRP, styles[gi]))

    with tile.TileContext(nc) as tc, ExitStack() as ctx:
        cpool = ctx.enter_context(tc.tile_pool(name="const", bufs=1))
        wpool = ctx.enter_context(tc.tile_pool(name="whx", bufs=1))
        spool = ctx.enter_context(tc.tile_pool(name="srelu", bufs=2))
        zpool = ctx.enter_context(tc.tile_pool(name="zmax", bufs=4))
        ppool = ctx.enter_context(tc.tile_pool(name="p3", bufs=4))
        opool = ctx.enter_context(tc.tile_pool(name="out", bufs=2))
        paq = ctx.enter_context(
            tc.tile_pool(name="psa", bufs=min(H, 8), space="PSUM")
        )

        # ---- resident constants -------------------------------------------
        # issue order matters: iw/niw/gB feed the first compute; the mask
        # parts and whx chunks stream behind.
        iw_t = cpool.tile([128, H * CJ], F32, tag="iwcol")
        nc.sync.dma_start(iw_t[:], iwcol[:])
        niw_t = cpool.tile([128, H * CJ], F32, tag="niwcol")
        nc.sync.dma_start(niw_t[:], niwcol[:])
        g_t = cpool.tile([128, H, R], BF, tag="gB")
        nc.sync.dma_start(g_t[:], gBin[:])

        whxa_t = cpool.tile([128, nA, H, WPHA], F8, tag="whxa")
        nc.sync.dma_start(whxa_t[:], whxa[:])
        m8_t = cpool.tile([128, CJ, R], F8, tag="mask8")
        NMQ = 4
        for mq in range(NMQ):
            cs = slice(mq * (CJ // NMQ), (mq + 1) * (CJ // NMQ))
            nc.sync.dma_start(m8_t[:, cs, :], mask8[:, cs, :])

        # interleave the bf16 mask parts and stationary parts so early
        # chunks of both land first
        mask_t = cpool.tile([128, CJ, R], BF, tag="mask")
        whx_t = wpool.tile([128, CJ, H, WPH], BF, tag="whxall")
        for mq in range(NMQ):
            cs = slice(mq * (CJ // NMQ), (mq + 1) * (CJ // NMQ))
            nc.sync.dma_start(mask_t[:, cs, :], maskM[:, cs, :])
            nc.sync.dma_start(whx_t[:, cs], whxin[:, cs])

        n_extra = sum(G // 2 for _, G, s in groups if s == "A")
        total_mm = CJ + n_extra

        # tt consumption order: P groups early (Pool gets going), A groups
        # spread mid (ScalarE results arrive while DVE does V work).
        p_g = [t for t in groups if t[2] == "P"]
        a_g = [t for t in groups if t[2] == "A"]
        v_g = [t for t in groups if t[2] == "V"]
        tt_order = []
        qs = [p_g, a_g, v_g]
        while any(qs):
            for q in qs:
                if q:
                    tt_order.append(q.pop(0))

        # ---- attention + aggregation --------------------------------------
        # One PSUM bank per head, all live at once.  Each head's A-style "+m"
        # matmuls (fp8 DoubleRow pairs: rhs [128, 2, R], lhsT [128, 2, HID+1]
        # contracting TWO mask chunks at half the row cost) issue at the head
        # start: they only need DMA-landed data, so they fill what would be
        # PE idle gaps between heads and keep the HAM throttle warm.
        pas = [
            paq.tile([HID + 1, R], F32, tag="psa", name=f"pa{h}")
            for h in range(H)
        ]

        for h in range(H):
            pa = pas[h]
            mm = 0
            for c0, G, sty in groups:
                if sty != "A":
                    continue
                for k in range(0, G, 2):
                    c = c0 + k
                    # weights AP [Ki, Ko=2, M]: pair at dim 1 with a 16B-
                    # aligned step (BIR + s3_lw dual-fp8 restrictions)
                    nc.tensor.matmul(
                        pa[:],
                        whxa_t[:, a_ix[c] : a_ix[c] + 2, h, 0 : HID + 1],
                        m8_t[:, c : c + 2, :],
                        start=(mm == 0), stop=False,
                        perf_mode=DROW,
                    )
                    mm += 1

            # A-style ScalarE tiles: s = relu(g_i - iw_j)
            s_tiles = {}
            for c0, G, sty in groups:
                if sty != "A":
                    continue
                s = spool.tile([128, GRP, R], BF, tag="sa")
                for k in range(G):
                    ix = h * CJ + c0 + k
                    nc.scalar.activation(
                        s[:, k, :], g_t[:, h, :], RELU,
                        bias=niw_t[:, ix : ix + 1], scale=1.0,
                    )
                s_tiles[c0] = s

            # z = max(g_i, iw_j): DVE tensor_scalar (V and P styles)
            z_tiles = {}
            for c0, G, sty in groups:
                if sty == "A":
                    continue
                z = zpool.tile([128, GRP, R], BF, tag="zm")
                for k in range(G):
                    ix = h * CJ + c0 + k
                    nc.vector.tensor_scalar(
                        z[:, k, :], g_t[:, h, :],
                        iw_t[:, ix : ix + 1], None, op0=MAX,
                    )
                z_tiles[c0] = z

            # mask multiplies (DVE grouped; GpSimd NEVER touches these --
            # concurrent Q7 SBUF traffic halves DVE throughput) + matmuls
            for c0, G, sty in tt_order:
                p3 = ppool.tile([128, GRP, R], BF, tag="p3")
                src = s_tiles[c0] if sty == "A" else z_tiles[c0]
                nc.vector.tensor_tensor(
                    p3[:, 0:G, :], src[:, 0:G, :],
                    mask_t[:, c0 : c0 + G, :], op=MUL,
                )
                for k in range(G):
                    c = c0 + k
                    nc.tensor.matmul(
                        pa[:], whx_t[:, c, h, 0 : HID + 1], p3[:, k, :],
                        start=(mm == 0), stop=(mm == total_mm - 1),
                    )
                    mm += 1

            o = opool.tile([HID + 1, R], F32, tag="aggo")
            nc.scalar.copy(o[:], pa[:])
            nc.sync.dma_start(agg[h], o[:])

    return nc


_PROGS = {}


def _get_prog(H, HID, styles):
    """Build (and cache) the layer program with the walrus wait-split fix
    applied.  The fix is HW-only: CoreSim's event loop rejects the injected
    NoOps, so sim users should call _build_layer directly."""
    key = (H, HID, styles)
    if key not in _PROGS:
        nc = _build_layer(H, HID, styles)
        _split_excess_waits(nc)
        _PROGS[key] = nc
    return _PROGS[key]


def _elu(v):
    return np.where(v > 0, v, np.expm1(np.minimum(v, 0.0))).astype(np.float32)


def _col_layout(x, H):
    """[N, H] -> [128, H*CJ] with [p, h*CJ+c] = x[128c+p, h]."""
    return np.ascontiguousarray(
        x.T.reshape(H, CJ, 128).transpose(2, 0, 1).reshape(128, H * CJ)
    ).astype(np.float32)


def _whx_layout(Wh, col_scale, H, HID, chunks=None, dtype=BF16, wph=None):
    """Stationary tensor: [128, CJ', H, WPH] with per-head [HID] features
    scaled by col_scale plus a col_scale denominator column."""
    WPH = wph if wph is not None else HID + 2
    scaled = (Wh.reshape(N, H, HID) * col_scale[:, :, None]).astype(np.float32)
    full = np.zeros((128, CJ, H, WPH), np.float32)
    full[:, :, :, :HID] = scaled.reshape(CJ, 128, H, HID).transpose(1, 0, 2, 3)
    full[:, :, :, HID] = col_scale.reshape(CJ, 128, H).transpose(1, 0, 2)
    if chunks is not None:
        full = full[:, chunks]
    return np.ascontiguousarray(full).astype(dtype)


def _host_inputs(f_src, f_dst, adj, Wh, H, styles):
    """Shared per-layer host prep.  f_src/f_dst [N, H] f32, adj [N, N] i32,
    Wh [N, H*HID] f32 (pre-activation per-head features)."""
    HID = Wh.shape[1] // H
    wpha = HID + 2
    while wpha % 16:
        wpha += 1
    iw = np.exp(0.8 * f_dst).astype(np.float32)       # [N, H]
    iw_arr = _col_layout(iw, H)

    ach = _a_chunks(styles)
    shared = {
        "iwcol": iw_arr,
        "niwcol": -iw_arr,
        # V/P stationaries: Wh * exp(.2 fd) (= Wh * v * w)
        "whxin": _whx_layout(Wh, np.exp(0.2 * f_dst).astype(np.float32), H, HID),
        # A-style "+m" stationaries: Wh * exp(fd) (= Wh * v), A-chunks only,
        # fp8 for DoubleRow paired matmuls (stride padded to 16B multiples)
        "whxa": _whx_layout(
            Wh, np.exp(f_dst).astype(np.float32), H, HID,
            chunks=ach if ach else [0], dtype=F8NP, wph=wpha,
        ),
    }
    g8 = np.exp(-0.8 * f_src).astype(np.float32)  # [N, H]
    per_core = []
    for i in range(NCORES):
        rows = slice(R * i, R * (i + 1))
        adjT = adj[rows, :].T.astype(np.float32)  # [N, R] 0/1
        d = dict(shared)
        mm_arr = np.ascontiguousarray(adjT.reshape(CJ, 128, R).transpose(1, 0, 2))
        d["maskM"] = mm_arr.astype(BF16)
        d["mask8"] = mm_arr.astype(F8NP)
        gs = np.ascontiguousarray(g8[rows, :].T)  # [H, R]
        d["gBin"] = np.broadcast_to(gs[None, :, :], (128, H, R)).astype(BF16)
        per_core.append(d)
    return per_core


def _run_layer(nc, in_maps, H, HID, tag):
    t0 = time.time()
    res = run_bass_kernel_spmd(nc, in_maps, core_ids=CORE_IDS)
    LAST_PERF[f"{tag}_wall_s"] = time.time() - t0
    LAST_PERF[f"{tag}_exec_ns"] = res.exec_time_ns

    hT = np.empty((H * HID, N), np.float32)
    for i in range(NCORES):
        a = res.results[i]["agg"]  # [H, HID+1, R]
        denom = a[:, HID : HID + 1, :]
        hT[:, R * i : R * (i + 1)] = (a[:, :HID, :] / denom).reshape(H * HID, R)
    return hT


def kernel(x, adj, W1, a1, W2, a2):
    x = np.asarray(x, np.float32)
    adj = np.asarray(adj, np.int32)
    W1 = np.asarray(W1, np.float32)
    a1 = np.asarray(a1, np.float32)
    W2 = np.asarray(W2, np.float32)
    a2 = np.asarray(a2, np.float32)

    H1, HID1, OUT = W1.shape[0], W1.shape[2], W2.shape[1]

    progA = _get_prog(H1, HID1, STY1)
    progB = _get_prog(1, OUT, STY2)

    # ---- layer 1 ----------------------------------------------------------
    W1c = np.ascontiguousarray(W1.transpose(1, 0, 2).reshape(FIN, H1 * HID1))
    wsrc1 = np.einsum("hfk,hk->fh", W1, a1[:, :HID1, 0]).astype(np.float32)
    wdst1 = np.einsum("hfk,hk->fh", W1, a1[:, HID1:, 0]).astype(np.float32)
    f_src1 = x @ wsrc1  # [N, H]
    f_dst1 = x @ wdst1
    Wh1 = x @ W1c  # [N, H1*HID1]

    in_maps = _host_inputs(f_src1, f_dst1, adj, Wh1, H1, STY1)
    hT = _run_layer(progA, in_maps, H1, HID1, "layer1")
    hcatT = _elu(hT)  # [512, N] == h_cat.T (concat=True applies elu)

    # ---- layer 2 ----------------------------------------------------------
    hcat = np.ascontiguousarray(hcatT.T)  # [N, 512]
    wsrc2 = (W2 @ a2[:OUT, 0]).astype(np.float32)[:, None]
    wdst2 = (W2 @ a2[OUT:, 0]).astype(np.float32)[:, None]
    f_src2 = hcat @ wsrc2  # [N, 1]
    f_dst2 = hcat @ wdst2
    Wh2 = hcat @ W2  # [N, OUT]
    in_maps2 = _host_inputs(f_src2, f_dst2, adj, Wh2, 1, STY2)
    outT = _run_layer(progB, in_maps2, 1, OUT, "layer2")
    # layer 2: concat=False -> no inner elu; final output = elu(out)
    return np.ascontiguousarray(_elu(outT).T)


# revision 49
# speedup vs baseline: 1.0621x; 1.0376x over previous
"""Trainium2 Bass kernel for a 2-layer dense-adjacency GAT (nn_GAT_17824114278677).

Sharding: nodes (rows of the attention matrix) are sharded across the 8
NeuronCores, 512 rows per core; weights and node features are replicated.
Two SPMD launches (one per GAT layer) with a host-side gather of the layer-1
output in between.

Per-core dataflow: attention tiles are computed TRANSPOSED, [j=128
partitions, r=512 rows], so the aggregation att @ Wh maps directly onto the
PE (contraction over j on partitions) with zero on-chip transposes.

Math: softmax is invariant to per-row scaling, so the row factor
u_i = exp(f_src_i) cancels; per-key factors fold into the stationary
operand.  From exp(leaky_relu(t)) = max(exp(t), exp(0.2 t)):

    att[j, i]  (up to a row factor)
      = v_j * m01[j, i] * max(1, g_i * w_j)        g = exp(-0.8 f_src)
      = (v_j w_j) * m01[j, i] * max(iw_j, g_i)     w = exp(-0.8 f_dst)
                                                   iw = 1/w, v = exp(f_dst)

so with the stationary Whx2 = Wh * exp(0.2 f_dst) (and an exp(0.2 f_dst)
denominator column), the per-element work is exactly

    z = max(g_i, iw_j)   (one tensor_scalar, op0=max, per-partition scalar)
    p = z * m01          (one tensor_tensor)

Masked entries are exactly 0, matching the reference's -9e15 mask.  Chunk
groups are routed per-style to balance the engines (HW-measured rates):

  V: DVE tensor_scalar z (~347ns/[128,512]) + DVE tensor_tensor p
     (~303ns/chunk grouped x4).
  A: ScalarE s = relu(g - iw) (per-partition bias AP, ~706ns) + DVE
     tensor_tensor p = s*m; the missing "+m" term rides the PE as fp8e4
     DoubleRow PAIRED matmuls of the raw mask against Whx1 = Wh*exp(f_dst)
     (m*max(iw,g)*w*v = m*v + m*relu(g-iw)*exp(.2 fd)); a pair of mask
     chunks costs 421ns (vs 379ns/chunk for normal bf16 matmuls).

GpSimd is left COMPLETELY idle on purpose: its Q7 software tensor ops
share SBUF ports with the DVE and degrade concurrent DVE throughput ~4x.
Per-head PSUM banks (all 8 live) let each head's mask-matmuls issue at
head start, filling PE idle windows so the HAM throttle stays warm.

Softmax denominators ride as an extra stationary column; division + ELU
happen on the host on the tiny per-head [HID+1, 512] outputs.

Measured on 8 axon-tunneled trn2 cores: ~140.5us (layer 1) + ~44.8us
(layer 2) = ~185.3us total, rel err ~5.9e-3 vs the fp32 jax reference
(fp8 mask-matmul terms cost ~5e-3; the bf16-only variant measures 8.3e-4
at ~264us).  Layer 1 runs with the PE at ~95% of span (379ns warm
matmuls), DVE ~92%, ScalarE ~74%.
"""

import os
import sys
import time
from contextlib import ExitStack

for _p in ("/opt/trn_rl_repo", "/root/.axon_site/_ro/trn_rl_repo"):
    if os.path.isdir(_p) and _p not in sys.path:
        sys.path.append(_p)

import numpy as np
import ml_dtypes

import bass_rust
import concourse.bass as bass
import concourse.tile as tile
from concourse import mybir
from concourse.bass_utils import run_bass_kernel_spmd

BF16 = ml_dtypes.bfloat16
F32 = mybir.dt.float32
BF = mybir.dt.bfloat16
F8 = mybir.dt.float8e4
F8NP = mybir.dt.np(F8)
DROW = mybir.MatmulPerfMode.DoubleRow

N = 4096          # nodes
NCORES = 8
R = N // NCORES   # rows (queries) per core
CJ = N // 128     # 32 key chunks
FIN = 512         # input feature dim of both layers
GRP = 4           # chunk-group size for fused VectorE/Pool ops

# Per-head style schedule for the CJ//GRP = 8 chunk groups.
# 'V' = DVE-only, 'A' = ScalarE relu + DVE mask (+ extra PE matmul),
# 'P' = DVE tensor_scalar + Pool(GpSimd) mask multiply.
STY1 = ("A", "V", "A", "V", "A", "V", "A", "V")   # layer 1
STY2 = ("A", "V", "V", "V", "A", "V", "V", "V")   # layer 2

CORE_IDS = list(range(NCORES))

LAST_PERF = {}


# ---------------------------------------------------------------------------
# walrus workaround: it rejects instructions carrying >1 sync-wait command
# ("Too many sync wait commands").  Move excess waits onto preceding
# same-engine NoOps -- semantically identical (same-engine waits are totally
# ordered before the instruction).
def _split_excess_waits(nc, max_waits: int = 1) -> int:
    n_split = 0
    for fn in nc.m.functions:
        for bb in fn.blocks:
            insts = bb.instructions
            new_insts = []
            changed = False
            for ins in insts:
                si = ins.sync_info
                waits = list(si.on_wait) if si is not None else []
                if len(waits) > max_waits:
                    extra, keep = waits[:-max_waits], waits[-max_waits:]
                    for k in range(0, len(extra), max_waits):
                        chunk = extra[k : k + max_waits]
                        nop = bass_rust.InstNoOp(
                            name=f"{ins.name}-wsplit{k}", ins=[], outs=[]
                        )
                        nop.engine = ins.engine
                        nop.sync_info = mybir.SyncInfo(on_wait=chunk, on_update=[])
                        new_insts.append(nop)
                        n_split += 1
                    si.on_wait = keep
                    changed = True
                new_insts.append(ins)
            if changed:
                bb.instructions = new_insts
    return n_split


def _a_chunks(styles):
    """Absolute chunk indices covered by 'A' style groups."""
    out = []
    for gi, s in enumerate(styles):
        if s == "A":
            out.extend(range(gi * GRP, gi * GRP + GRP))
    return out


# ---------------------------------------------------------------------------
def _build_layer(H: int, HID: int, styles=STY1, split: int = 1):
    """One GAT layer, per-core program.

    Inputs (per core):
      whxin  [128, CJ, H, WPH] bf16  Wh*exp(.2 fd) per head + exp(.2 fd) col
      whxa   [128, nA, H, WPH] bf16  Wh*exp(fd) + exp(fd) col, A-chunks only
      maskM  [128, CJ, R]      bf16  0/1 adjacency, chunk-major, transposed
      gBin   [128, H, R]       bf16  exp(-0.8 f_src) of this core's rows,
                                     broadcast along partitions
      iwcol  [128, H*CJ]       f32   exp(0.8 f_dst), [p, h*CJ+c] = iw[h, 128c+p]
      niwcol [128, H*CJ]       f32   -exp(0.8 f_dst)
    Output:
      agg    [H, HID+1, R]  f32   rows 0..HID-1: unnormalized transposed
                                  numerator; row HID: softmax denominator
    """
    WPH = HID + 2  # per-head stride in whx: HID cols + denom col + pad
    # whxa per-head stride padded to a 16-byte multiple so every per-head
    # slice offset and the DoubleRow pair step meet the s3_lw dual-fp8
    # alignment restrictions
    WPHA = HID + 2
    while WPHA % 16:
        WPHA += 1
    ach = _a_chunks(styles)
    a_ix = {c: i for i, c in enumerate(ach)}
    nA = max(1, len(ach))

    nc = bass.Bass("TRN2", debug=False, num_devices=NCORES)
    whxin = nc.dram_tensor("whxin", [128, CJ, H, WPH], BF, kind="ExternalInput")
    whxa = nc.dram_tensor("whxa", [128, nA, H, WPHA], F8, kind="ExternalInput")
    maskM = nc.dram_tensor("maskM", [128, CJ, R], BF, kind="ExternalInput")
    mask8 = nc.dram_tensor("mask8", [128, CJ, R], F8, kind="ExternalInput")
    gBin = nc.dram_tensor("gBin", [128, H, R], BF, kind="ExternalInput")
    iwcol = nc.dram_tensor("iwcol", [128, H * CJ], F32, kind="ExternalInput")
    niwcol = nc.dram_tensor("niwcol", [128, H * CJ], F32, kind="ExternalInput")
    # `split` > 1 (only for H == 1) carves the single head into `split`
    # virtual passes over disjoint key-chunk ranges, each with its own PSUM
    # bank, restoring the cross-pass engine pipelining that multi-head
    # layers get for free.  The host sums the partial numerators and
    # denominators.
    NP = H * split
    agg = nc.dram_tensor("agg", [NP, HID + 1, R], F32, kind="ExternalOutput")

    RELU = mybir.ActivationFunctionType.Relu
    MAX = mybir.AluOpType.max
    MUL = mybir.AluOpType.mult

    groups = []
    for gi in range(CJ // GRP):
        groups.append((gi * GRP, GRP, styles[gi]))

    with tile.TileContext(nc) as tc, ExitStack() as ctx:
        cpool = ctx.enter_context(tc.tile_pool(name="const", bufs=1))
        wpool = ctx.enter_context(tc.tile_pool(name="whx", bufs=1))
        spool = ctx.enter_context(tc.tile_pool(name="srelu", bufs=2))
        zpool = ctx.enter_context(tc.tile_pool(name="zmax", bufs=4))
        ppool = ctx.enter_context(tc.tile_pool(name="p3", bufs=4))
        opool = ctx.enter_context(tc.tile_pool(name="out", bufs=2))
        paq = ctx.enter_context(
            tc.tile_pool(name="psa", bufs=min(NP, 8), space="PSUM")
        )

        # ---- resident constants -------------------------------------------
        # issue order matters: iw/niw/gB feed the first compute; the mask
        # parts and whx chunks stream behind.
        iw_t = cpool.tile([128, H * CJ], F32, tag="iwcol")
        nc.sync.dma_start(iw_t[:], iwcol[:])
        niw_t = cpool.tile([128, H * CJ], F32, tag="niwcol")
        nc.sync.dma_start(niw_t[:], niwcol[:])
        g_t = cpool.tile([128, H, R], BF, tag="gB")
        nc.sync.dma_start(g_t[:], gBin[:])

        whxa_t = cpool.tile([128, nA, H, WPHA], F8, tag="whxa")
        nc.sync.dma_start(whxa_t[:], whxa[:])
        m8_t = cpool.tile([128, CJ, R], F8, tag="mask8")
        NMQ = 4
        for mq in range(NMQ):
            cs = slice(mq * (CJ // NMQ), (mq + 1) * (CJ // NMQ))
            nc.sync.dma_start(m8_t[:, cs, :], mask8[:, cs, :])

        # interleave the bf16 mask parts and stationary parts so early
        # chunks of both land first
        mask_t = cpool.tile([128, CJ, R], BF, tag="mask")
        whx_t = wpool.tile([128, CJ, H, WPH], BF, tag="whxall")
        for mq in range(NMQ):
            cs = slice(mq * (CJ // NMQ), (mq + 1) * (CJ // NMQ))
            nc.sync.dma_start(mask_t[:, cs, :], maskM[:, cs, :])
            nc.sync.dma_start(whx_t[:, cs], whxin[:, cs])

        # one pass per (head x chunk-range); L1 has one full-CJ pass per head
        gpp = len(groups) // split
        passes = []
        for h in range(H):
            for sp in range(split):
                passes.append((h, groups[sp * gpp : (sp + 1) * gpp]))

        def _tt_order(pgroups):
            # A groups interleaved with V: ScalarE results arrive while the
            # DVE works through V groups
            a_g = [t for t in pgroups if t[2] == "A"]
            v_g = [t for t in pgroups if t[2] == "V"]
            order = []
            qs = [a_g, v_g]
            while any(qs):
                for q in qs:
                    if q:
                        order.append(q.pop(0))
            return order

        # ---- attention + aggregation --------------------------------------
        # One PSUM bank per pass, all live at once.  Each pass's A-style "+m"
        # matmuls (fp8 DoubleRow pairs: rhs [128, 2, R], lhsT [128, 2, HID+1]
        # contracting TWO mask chunks at half the row cost) issue at the pass
        # start: they only need DMA-landed data, so they fill what would be
        # PE idle gaps between passes and keep the HAM throttle warm.
        pas = [
            paq.tile([HID + 1, R], F32, tag="psa", name=f"pa{pi}")
            for pi in range(NP)
        ]

        for pi, (h, pgroups) in enumerate(passes):
            pa = pas[pi]
            mm = 0
            total_mm = sum(G for _, G, _ in pgroups) + sum(
                G // 2 for _, G, s in pgroups if s == "A"
            )
            for c0, G, sty in pgroups:
                if sty != "A":
                    continue
                for k in range(0, G, 2):
                    c = c0 + k
                    # weights AP [Ki, Ko=2, M]: pair at dim 1 with a 16B-
                    # aligned step (BIR + s3_lw dual-fp8 restrictions)
                    nc.tensor.matmul(
                        pa[:],
                        whxa_t[:, a_ix[c] : a_ix[c] + 2, h, 0 : HID + 1],
                        m8_t[:, c : c + 2, :],
                        start=(mm == 0), stop=False,
                        perf_mode=DROW,
                    )
                    mm += 1

            # A-style ScalarE tiles: s = relu(g_i - iw_j)
            s_tiles = {}
            for c0, G, sty in pgroups:
                if sty != "A":
                    continue
                s = spool.tile([128, GRP, R], BF, tag="sa")
                for k in range(G):
                    ix = h * CJ + c0 + k
                    nc.scalar.activation(
                        s[:, k, :], g_t[:, h, :], RELU,
                        bias=niw_t[:, ix : ix + 1], scale=1.0,
                    )
                s_tiles[c0] = s

            # z = max(g_i, iw_j): DVE tensor_scalar (V style)
            z_tiles = {}
            for c0, G, sty in pgroups:
                if sty == "A":
                    continue
                z = zpool.tile([128, GRP, R], BF, tag="zm")
                for k in range(G):
                    ix = h * CJ + c0 + k
                    nc.vector.tensor_scalar(
                        z[:, k, :], g_t[:, h, :],
                        iw_t[:, ix : ix + 1], None, op0=MAX,
                    )
                z_tiles[c0] = z

            # mask multiplies (DVE grouped; GpSimd NEVER touches these --
            # concurrent Q7 SBUF traffic halves DVE throughput) + matmuls
            for c0, G, sty in _tt_order(pgroups):
                p3 = ppool.tile([128, GRP, R], BF, tag="p3")
                src = s_tiles[c0] if sty == "A" else z_tiles[c0]
                nc.vector.tensor_tensor(
                    p3[:, 0:G, :], src[:, 0:G, :],
                    mask_t[:, c0 : c0 + G, :], op=MUL,
                )
                for k in range(G):
                    c = c0 + k
                    nc.tensor.matmul(
                        pa[:], whx_t[:, c, h, 0 : HID + 1], p3[:, k, :],
                        start=(mm == 0), stop=(mm == total_mm - 1),
                    )
                    mm += 1

            o = opool.tile([HID + 1, R], F32, tag="aggo")
            nc.scalar.copy(o[:], pa[:])
            nc.sync.dma_start(agg[pi], o[:])

    return nc


_PROGS = {}


def _get_prog(H, HID, styles, split=1):
    """Build (and cache) the layer program with the walrus wait-split fix
    applied.  The fix is HW-only: CoreSim's event loop rejects the injected
    NoOps, so sim users should call _build_layer directly."""
    key = (H, HID, styles, split)
    if key not in _PROGS:
        nc = _build_layer(H, HID, styles, split)
        _split_excess_waits(nc)
        _PROGS[key] = nc
    return _PROGS[key]


def _elu(v):
    return np.where(v > 0, v, np.expm1(np.minimum(v, 0.0))).astype(np.float32)


def _col_layout(x, H):
    """[N, H] -> [128, H*CJ] with [p, h*CJ+c] = x[128c+p, h]."""
    return np.ascontiguousarray(
        x.T.reshape(H, CJ, 128).transpose(2, 0, 1).reshape(128, H * CJ)
    ).astype(np.float32)


def _whx_layout(Wh, col_scale, H, HID, chunks=None, dtype=BF16, wph=None):
    """Stationary tensor: [128, CJ', H, WPH] with per-head [HID] features
    scaled by col_scale plus a col_scale denominator column."""
    WPH = wph if wph is not None else HID + 2
    scaled = (Wh.reshape(N, H, HID) * col_scale[:, :, None]).astype(np.float32)
    full = np.zeros((128, CJ, H, WPH), np.float32)
    full[:, :, :, :HID] = scaled.reshape(CJ, 128, H, HID).transpose(1, 0, 2, 3)
    full[:, :, :, HID] = col_scale.reshape(CJ, 128, H).transpose(1, 0, 2)
    if chunks is not None:
        full = full[:, chunks]
    return np.ascontiguousarray(full).astype(dtype)


def _host_inputs(f_src, f_dst, adj, Wh, H, styles):
    """Shared per-layer host prep.  f_src/f_dst [N, H] f32, adj [N, N] i32,
    Wh [N, H*HID] f32 (pre-activation per-head features)."""
    HID = Wh.shape[1] // H
    wpha = HID + 2
    while wpha % 16:
        wpha += 1
    iw = np.exp(0.8 * f_dst).astype(np.float32)       # [N, H]
    iw_arr = _col_layout(iw, H)

    ach = _a_chunks(styles)
    shared = {
        "iwcol": iw_arr,
        "niwcol": -iw_arr,
        # V/P stationaries: Wh * exp(.2 fd) (= Wh * v * w)
        "whxin": _whx_layout(Wh, np.exp(0.2 * f_dst).astype(np.float32), H, HID),
        # A-style "+m" stationaries: Wh * exp(fd) (= Wh * v), A-chunks only,
        # fp8 for DoubleRow paired matmuls (stride padded to 16B multiples)
        "whxa": _whx_layout(
            Wh, np.exp(f_dst).astype(np.float32), H, HID,
            chunks=ach if ach else [0], dtype=F8NP, wph=wpha,
        ),
    }
    g8 = np.exp(-0.8 * f_src).astype(np.float32)  # [N, H]
    per_core = []
    for i in range(NCORES):
        rows = slice(R * i, R * (i + 1))
        adjT = adj[rows, :].T.astype(np.float32)  # [N, R] 0/1
        d = dict(shared)
        mm_arr = np.ascontiguousarray(adjT.reshape(CJ, 128, R).transpose(1, 0, 2))
        d["maskM"] = mm_arr.astype(BF16)
        d["mask8"] = mm_arr.astype(F8NP)
        gs = np.ascontiguousarray(g8[rows, :].T)  # [H, R]
        d["gBin"] = np.broadcast_to(gs[None, :, :], (128, H, R)).astype(BF16)
        per_core.append(d)
    return per_core


def _run_layer(nc, in_maps, H, HID, tag, split=1):
    t0 = time.time()
    res = run_bass_kernel_spmd(nc, in_maps, core_ids=CORE_IDS)
    LAST_PERF[f"{tag}_wall_s"] = time.time() - t0
    LAST_PERF[f"{tag}_exec_ns"] = res.exec_time_ns

    hT = np.empty((H * HID, N), np.float32)
    for i in range(NCORES):
        a = res.results[i]["agg"]  # [H*split, HID+1, R]
        if split > 1:
            a = a.reshape(H, split, HID + 1, R).sum(axis=1)
        denom = a[:, HID : HID + 1, :]
        hT[:, R * i : R * (i + 1)] = (a[:, :HID, :] / denom).reshape(H * HID, R)
    return hT


def kernel(x, adj, W1, a1, W2, a2):
    x = np.asarray(x, np.float32)
    adj = np.asarray(adj, np.int32)
    W1 = np.asarray(W1, np.float32)
    a1 = np.asarray(a1, np.float32)
    W2 = np.asarray(W2, np.float32)
    a2 = np.asarray(a2, np.float32)

    H1, HID1, OUT = W1.shape[0], W1.shape[2], W2.shape[1]

    progA = _get_prog(H1, HID1, STY1)
    progB = _get_prog(1, OUT, STY2, split=2)

    # ---- layer 1 ----------------------------------------------------------
    W1c = np.ascontiguousarray(W1.transpose(1, 0, 2).reshape(FIN, H1 * HID1))
    wsrc1 = np.einsum("hfk,hk->fh", W1, a1[:, :HID1, 0]).astype(np.float32)
    wdst1 = np.einsum("hfk,hk->fh", W1, a1[:, HID1:, 0]).astype(np.float32)
    f_src1 = x @ wsrc1  # [N, H]
    f_dst1 = x @ wdst1
    Wh1 = x @ W1c  # [N, H1*HID1]

    in_maps = _host_inputs(f_src1, f_dst1, adj, Wh1, H1, STY1)
    hT = _run_layer(progA, in_maps, H1, HID1, "layer1")
    hcatT = _elu(hT)  # [512, N] == h_cat.T (concat=True applies elu)

    # ---- layer 2 ----------------------------------------------------------
    hcat = np.ascontiguousarray(hcatT.T)  # [N, 512]
    wsrc2 = (W2 @ a2[:OUT, 0]).astype(np.float32)[:, None]
    wdst2 = (W2 @ a2[OUT:, 0]).astype(np.float32)[:, None]
    f_src2 = hcat @ wsrc2  # [N, 1]
    f_dst2 = hcat @ wdst2
    Wh2 = hcat @ W2  # [N, OUT]
    in_maps2 = _host_inputs(f_src2, f_dst2, adj, Wh2, 1, STY2)
    outT = _run_layer(progB, in_maps2, 1, OUT, "layer2", split=2)
    # layer 2: concat=False -> no inner elu; final output = elu(out)
    return np.ascontiguousarray(_elu(outT).T)
